# revision 19
# baseline (speedup 1.0000x reference)
"""Multi-head causal attention (b=2, s=2048, d=1024, h=16) on 8 TRN2 cores.

Sharding: batch (2) x head-groups (4 heads each) -> 8 cores, Megatron-style.
Each core: QKV col-sliced projections (d -> 256), causal attention for its 4
heads, row-sliced output projection producing a partial [2048, 1024] output.
Host sums the 4 partials per batch and adds the output bias.

v2 design (fused streaming pipeline):
  - One pass over s in 4 chunks of 512: projections for chunk sc are emitted
    interleaved with attention for query block sc-1, so PE, ACT (exp), DVE
    and GPSIMD stay busy end-to-end and the PE HAM clock never re-throttles.
  - Heads processed in PAIRS stacked on partition halves: q/k projections are
    unpadded [d,128] blocks (half the baseline's PE work), scores for the two
    heads of a pair run CONCURRENTLY as row-tiled K=64 matmuls (tile_position
    (0,0)/(64,0)), halving effective scores time.
  - The two concurrent score matmuls write one [128, 2, 512] PSUM supertile
    (2 banks) so a single ACT exp instruction covers both heads, halving the
    per-instruction ACT overhead (352 cycles/instr).
  - Causal mask via gpsimd.affine_select on the exp output (no tri tensor,
    no mask DMA); diagonal o-offsets clamped to <=256 so every fp32r matmul
    keeps moving dim >= 256 (below that fp32r runs 4x slower).
  - Softmax denominator from an extra ones-column in v (row 64 of ctx PSUM),
    broadcast via gpsimd.partition_broadcast (no PE broadcast matmul).
  - All inputs host-packed so every DMA is a contiguous per-partition blob:
    4 weight DMAs + 11 x DMAs + 32 output DMAs total.
"""

import numpy as np

import concourse.bass as bass
import concourse.tile as tile
from concourse import bacc
from concourse import mybir
from concourse import bass_utils

F32 = mybir.dt.float32
F32R = mybir.dt.float32r
EXP = mybir.ActivationFunctionType.Exp

B, S, D, H = 2, 2048, 1024, 16
HG = 4                  # heads per core
E = 64                  # head dim
DG = HG * E             # 256, d-slice per core
NC = 8                  # cores
IT = 512                # query block (moving dim of attention matmuls)
JT = 128                # key tile
KC = D // 128           # 8 contraction chunks for projections
NSC = S // IT           # 4 s-chunks of 512
SCALE = 1.0 / np.sqrt(E)
SK = 2                  # ctx-matmul skew (att pipeline depth, in units)

_CACHE = {}


def _build():
    nc = bacc.Bacc("TRN2", target_bir_lowering=False, debug=False)

    xp = nc.dram_tensor("xp", [128, NSC * KC * IT], F32R, kind="ExternalInput").ap()
    wq = nc.dram_tensor("wq", [128, KC * DG], F32R, kind="ExternalInput").ap()
    wk = nc.dram_tensor("wk", [128, KC * DG], F32R, kind="ExternalInput").ap()
    wv = nc.dram_tensor("wv", [128, KC * DG], F32R, kind="ExternalInput").ap()
    wo = nc.dram_tensor("wo", [128, 2 * D], F32R, kind="ExternalInput").ap()
    on = nc.dram_tensor("on", [128, 128], F32R, kind="ExternalInput").ap()
    out = nc.dram_tensor("out", [S, D], F32, kind="ExternalOutput").ap()

    with tile.TileContext(nc) as tc:
        from contextlib import ExitStack

        with ExitStack() as ctx:
            pers = ctx.enter_context(tc.tile_pool(name="pers", bufs=1))

            wq_sb = pers.tile([128, KC * DG], F32R, tag="wq")         # 8 KB/p
            wk_sb = pers.tile([128, KC * DG], F32R, tag="wk")
            wv_sb = pers.tile([128, KC * DG], F32R, tag="wv")
            wo_sb = pers.tile([128, 2 * D], F32R, tag="wo")
            qT_sb = pers.tile([128, 2 * S], F32R, tag="qT")           # 16 KB/p
            kT_sb = pers.tile([128, 2 * S], F32R, tag="kT")
            v_sb = pers.tile([128, 16 * HG * (E + 1)], F32R, tag="v")  # 16.25 KB/p
            cx_sb = pers.tile([128, 2 * S], F32R, tag="cx")

            v4 = v_sb.rearrange("p (t h x) -> p t h x", t=16, h=HG)
            VW = HG * (E + 1)  # 260

            xs_pool = ctx.enter_context(tc.tile_pool(name="xsp", bufs=3))
            pt_pool = ctx.enter_context(tc.tile_pool(name="ptp", bufs=3))
            ot_pool = ctx.enter_context(tc.tile_pool(name="otp", bufs=2))
            dn_pool = ctx.enter_context(tc.tile_pool(name="dnp", bufs=2))
            sp_pool = ctx.enter_context(tc.tile_pool(name="spp", bufs=2, space="PSUM"))
            cp_pool = ctx.enter_context(tc.tile_pool(name="cpp", bufs=2, space="PSUM"))
            pj_pool = ctx.enter_context(tc.tile_pool(name="pjp", bufs=1, space="PSUM"))
            op_pool = ctx.enter_context(tc.tile_pool(name="opp", bufs=1, space="PSUM"))

            zero_reg = nc.gpsimd.to_reg(0.0)

            # ---- prologue DMAs (all contiguous; first chunks split out so
            # the first matmul can start after ~2 small transfers) ----
            nc.sync.dma_start(wq_sb[:, 0:DG], wq[:, 0:DG])
            nc.sync.dma_start(wk_sb[:, 0:DG], wk[:, 0:DG])
            xs_tiles = []
            xs0 = xs_pool.tile([128, KC, IT], F32R, tag="xs", name="xs0")
            xs_tiles.append(xs0)
            for k in range(KC):
                nc.sync.dma_start(xs0[:, k, :], xp[:, k * IT:(k + 1) * IT])
            nc.sync.dma_start(wq_sb[:, DG:], wq[:, DG:])
            nc.sync.dma_start(wk_sb[:, DG:], wk[:, DG:])
            nc.sync.dma_start(wv_sb[:], wv[:])
            xs1 = xs_pool.tile([128, KC, IT], F32R, tag="xs", name="xs1")
            xs_tiles.append(xs1)
            nc.sync.dma_start(xs1[:], xp[:, KC * IT:2 * KC * IT])
            nc.sync.dma_start(wo_sb[:], wo[:])
            # ones column of v (softmax denominator rides the ctx matmul)
            ones_sb = pers.tile([128, 128], F32R, tag="ones")
            nc.sync.dma_start(ones_sb[:], on[:])
            nc.vector.tensor_copy(
                v4[:, :, :, E], ones_sb[:, 0:64].rearrange("p (t h) -> p t h", h=HG))

            # ---- attention pipeline state machine ----
            ctx_q = []        # (emit_fn, group_end_fn | None)
            op_q = []         # pending out-proj emitters (prev query block)
            norms_open = [0]  # cx writes not yet emitted

            def tick():
                if op_q and norms_open[0] == 0:
                    op_q.pop(0)()

            def drain_ctx():
                emit, group_end = ctx_q.pop(0)
                emit()
                if group_end is not None:
                    group_end()

            cps_cur = {}      # head e -> cps tile for the pair in flight

            def emit_norm(cps, p, e, ti):
                # den row (part 64 of cps) -> K=1 PE broadcast matmul into the
                # proj psum slot -> recip -> scale ctx. Emitted inline at
                # group-end so the cps bank's next writer (the following
                # pair's first ctx matmul, drained one unit later) is emitted
                # after this read. (gpsimd.partition_broadcast would force a
                # GPSIMD microcode library swap against affine_select: ~8us.)
                dnr = dn_pool.tile([1, IT], F32R, tag="dnr")
                nc.vector.tensor_copy(dnr[:, :], cps[E:E + 1, :])
                dnb = pj_pool.tile([128, IT], F32, name="dnb", tag="pj")
                nc.tensor.matmul(dnb[:], lhsT=ones_sb[0:1, :], rhs=dnr[:, :],
                                 start=True, stop=True)
                rc = dn_pool.tile([64, IT], F32, tag="rc")
                nc.vector.reciprocal_approx_fast(rc[:, :], dnb[0:64, :])
                nc.vector.tensor_mul(
                    cx_sb[e * E:(e + 1) * E, p * S + ti * IT: p * S + (ti + 1) * IT],
                    cps[0:E, :], rc[:, :])
                norms_open[0] -= 1

            def att_unit(ti, p, jj, njt):
                def go():
                    d = jj - 4 * ti
                    o = min(max(d, 0) * JT, 256)
                    n = IT - o
                    sp = sp_pool.tile([128, 2, IT], F32, tag="sp")
                    for e in range(2):
                        # row-tiled K=64 pair: e=0 rows 0:64 / e=1 rows 64:128
                        # of the PE array run concurrently
                        nc.tensor.matmul(
                            sp[:, e, o:IT],
                            lhsT=kT_sb[e * E:(e + 1) * E, p * S + jj * JT: p * S + jj * JT + JT],
                            rhs=qT_sb[e * E:(e + 1) * E, p * S + ti * IT + o: p * S + (ti + 1) * IT],
                            start=True, stop=True,
                        )
                    pt = pt_pool.tile([128, 2, IT], F32R, tag="pt")
                    nc.scalar.activation(pt[:, :, o:IT], sp[:, :, o:IT], EXP, scale=SCALE)
                    if d >= 0:
                        # keep where query_pos >= key_pos
                        nc.gpsimd.affine_select(
                            pt[:, :, o:IT], pt[:, :, o:IT],
                            pattern=[[0, 2], [1, n]],
                            compare_op=mybir.AluOpType.is_ge,
                            fill=zero_reg,
                            base=ti * IT + o - jj * JT,
                            channel_multiplier=-1,
                        )

                    def emit_ctx(pt=pt, o=o, jj=jj, njt=njt):
                        if jj == 0:
                            for e in range(2):
                                cps_cur[e] = cp_pool.tile([128, IT], F32,
                                                          name="cps", tag="cps")
                        for e in range(2):
                            nc.tensor.matmul(
                                cps_cur[e][0:E + 1, o:IT],
                                lhsT=v_sb[:, jj * VW + (2 * p + e) * (E + 1):
                                          jj * VW + (2 * p + e + 1) * (E + 1)],
                                rhs=pt[:, e, o:IT],
                                start=(jj == 0), stop=(jj == njt - 1),
                            )
                    group_end = None
                    if jj == njt - 1:
                        norms_open[0] += 2
                        def group_end(p=p, ti=ti):
                            for e in range(2):
                                emit_norm(cps_cur[e], p, e, ti)
                    ctx_q.append((emit_ctx, group_end))
                    if len(ctx_q) > SK:
                        drain_ctx()
                    tick()
                return go

            def emit_op(ti):
                # for the final block the proj pool is idle: alternate into it
                # so out-proj matmuls overlap the staging copies
                def go_all():
                    for it_ in range(4 * ti, 4 * ti + 4):
                        def go(it_=it_):
                            ot = ot_pool.tile([128, 2 * IT], F32, tag="ott")
                            for dc in range(2):
                                pool = pj_pool if (ti == NSC - 1 and dc == 1) else op_pool
                                tag = "pj" if (ti == NSC - 1 and dc == 1) else "ops"
                                ps = pool.tile([128, IT], F32, name="ops", tag=tag)
                                for pr in range(2):
                                    nc.tensor.matmul(
                                        ps[:],
                                        lhsT=cx_sb[:, pr * S + it_ * JT: pr * S + it_ * JT + JT],
                                        rhs=wo_sb[:, pr * D + dc * IT: pr * D + (dc + 1) * IT],
                                        start=(pr == 0), stop=(pr == 1),
                                    )
                                nc.vector.tensor_copy(ot[:, dc * IT:(dc + 1) * IT], ps[:])
                            nc.sync.dma_start(out[it_ * JT:(it_ + 1) * JT, :], ot[:])
                        op_q.append(go)
                return go_all

            # ---- projection emitters ----
            def qk_group(sc, pair, which):
                def go():
                    ps = pj_pool.tile([128, IT], F32, name="pjt", tag="pj")
                    w_sb = wq_sb if which == "q" else wk_sb
                    for k in range(KC):
                        nc.tensor.matmul(
                            ps[:],
                            lhsT=w_sb[:, k * DG + pair * 128: k * DG + (pair + 1) * 128],
                            rhs=xs_tiles[sc][:, k, :],
                            start=(k == 0), stop=(k == KC - 1),
                        )
                    dst = qT_sb if which == "q" else kT_sb
                    nc.vector.tensor_copy(
                        dst[:, pair * S + sc * IT: pair * S + (sc + 1) * IT], ps[:])
                return go

            def v_group(sc, st):
                def go():
                    ps = pj_pool.tile([128, IT], F32, name="pjt", tag="pj")
                    for k in range(KC):
                        nc.tensor.matmul(
                            ps[:, 0:DG],
                            lhsT=xs_tiles[sc][:, k, st * JT:(st + 1) * JT],
                            rhs=wv_sb[:, k * DG:(k + 1) * DG],
                            start=(k == 0), stop=(k == KC - 1),
                        )
                    nc.vector.tensor_copy(
                        v4[:, sc * 4 + st, :, 0:E],
                        ps[:, 0:DG].rearrange("p (h e) -> p h e", e=E))
                return go

            # ---- phase driver ----
            # Phase ti: projections for chunk ti AND attention for query
            # block ti (its off-diagonal units only need previous chunks'
            # k/v; diagonal units follow this chunk's k/v groups). Every
            # phase, including the last, has proj matmuls to fill the PE
            # while ACT chews on exp supertiles.
            def merge(groups, units, gcyc, ucyc):
                tp, tu = len(groups) * gcyc, len(units) * ucyc
                pc = uc = 0
                while groups or units:
                    if groups and (not units or pc * tu <= uc * tp):
                        groups.pop(0)()
                        pc += gcyc
                    else:
                        units.pop(0)()
                        uc += ucyc

            for ti in range(NSC):
                sc = ti
                if sc + 2 < NSC:
                    xs_n = xs_pool.tile([128, KC, IT], F32R, tag="xs",
                                        name=f"xs{sc + 2}")
                    xs_tiles.append(xs_n)
                    nc.sync.dma_start(
                        xs_n[:], xp[:, (sc + 2) * KC * IT:(sc + 3) * KC * IT])
                njt = 4 * (ti + 1)
                qk_group(sc, 0, "q")()
                qk_group(sc, 0, "k")()
                # pair0 off-diagonal units interleaved with remaining groups
                rest = [qk_group(sc, 1, "q"), qk_group(sc, 1, "k"),
                        v_group(sc, 0), v_group(sc, 1),
                        v_group(sc, 2), v_group(sc, 3)]
                p0_off = [att_unit(ti, 0, jj, njt) for jj in range(4 * ti)]
                merge(rest, p0_off, 3072, 1536)
                for jj in range(4 * ti, njt):          # pair0 diagonal
                    att_unit(ti, 0, jj, njt)()
                for jj in range(njt):                  # pair1
                    att_unit(ti, 1, jj, njt)()
                emit_op(ti)()

            while ctx_q:
                drain_ctx()
                tick()
            for _ in range(80):
                if not op_q:
                    break
                tick()
            assert not op_q and norms_open[0] == 0

    nc.compile()
    return nc


def _pack_x(xb):
    # x[b] [2048, 1024] -> [128, 4*8*512]: chunk (sc, k) = xT[k*128:+128, sc*512:+512]
    return np.ascontiguousarray(
        xb.reshape(NSC, IT, KC, 128).transpose(3, 0, 2, 1).reshape(128, NSC * KC * IT))


def _pack_w(w):
    # [1024, 256] -> [128, 8*256] chunk-major
    return np.ascontiguousarray(
        w.reshape(KC, 128, DG).transpose(1, 0, 2).reshape(128, KC * DG))


def _pack_wo(w):
    # [256, 1024] -> [128, 2*1024] pair-major
    return np.ascontiguousarray(
        w.reshape(2, 128, D).transpose(1, 0, 2).reshape(128, 2 * D))


def _in_maps(x, Wq, Wk, Wv, Wo):
    maps = []
    for c in range(NC):
        b, g = c // (NC // B), c % (NC // B)
        maps.append({
            "xp": _pack_x(x[b]),
            "wq": _pack_w(Wq[:, g * DG:(g + 1) * DG]),
            "wk": _pack_w(Wk[:, g * DG:(g + 1) * DG]),
            "wv": _pack_w(Wv[:, g * DG:(g + 1) * DG]),
            "wo": _pack_wo(Wo[g * DG:(g + 1) * DG, :]),
            "on": np.ones((128, 128), np.float32),
        })
    return maps


def run(x, Wq, Wk, Wv, Wo, bo, trace=False):
    if "nc" not in _CACHE:
        _CACHE["nc"] = _build()
    nc = _CACHE["nc"]
    res = bass_utils.run_bass_kernel_spmd(
        nc, _in_maps(x, Wq, Wk, Wv, Wo), core_ids=list(range(NC)), trace=trace,
    )
    parts = [res.results[c]["out"] for c in range(NC)]
    gpb = NC // B
    full = np.stack([sum(parts[b * gpb + 1: (b + 1) * gpb], parts[b * gpb]) for b in range(B)])
    full = full + np.asarray(bo, np.float32)[None, None, :]
    return full.astype(np.float32), res


def kernel(x, Wq, Wk, Wv, Wo, bo):
    x = np.asarray(x, np.float32)
    full, _ = run(x, np.asarray(Wq, np.float32), np.asarray(Wk, np.float32),
                  np.asarray(Wv, np.float32), np.asarray(Wo, np.float32),
                  np.asarray(bo, np.float32))
    return full


# revision 20
# speedup vs baseline: 1.0316x; 1.0316x over previous
"""Multi-head causal attention (b=2, s=2048, d=1024, h=16) on 8 TRN2 cores.

Sharding: batch (2) x head-groups (4 heads each) -> 8 cores, Megatron-style.
Each core: QKV col-sliced projections (d -> 256), causal attention for its 4
heads, row-sliced output projection producing a partial [2048, 1024] output.
Host sums the 4 partials per batch and adds the output bias.

v2 design (fused streaming pipeline):
  - One pass over s in 4 chunks of 512: projections for chunk sc are emitted
    interleaved with attention for query block sc-1, so PE, ACT (exp), DVE
    and GPSIMD stay busy end-to-end and the PE HAM clock never re-throttles.
  - Heads processed in PAIRS stacked on partition halves: q/k projections are
    unpadded [d,128] blocks (half the baseline's PE work), scores for the two
    heads of a pair run CONCURRENTLY as row-tiled K=64 matmuls (tile_position
    (0,0)/(64,0)), halving effective scores time.
  - The two concurrent score matmuls write one [128, 2, 512] PSUM supertile
    (2 banks) so a single ACT exp instruction covers both heads, halving the
    per-instruction ACT overhead (352 cycles/instr).
  - Causal mask via gpsimd.affine_select on the exp output (no tri tensor,
    no mask DMA); diagonal o-offsets clamped to <=256 so every fp32r matmul
    keeps moving dim >= 256 (below that fp32r runs 4x slower).
  - Softmax denominator from an extra ones-column in v (row 64 of ctx PSUM),
    broadcast via gpsimd.partition_broadcast (no PE broadcast matmul).
  - All inputs host-packed so every DMA is a contiguous per-partition blob:
    4 weight DMAs + 11 x DMAs + 32 output DMAs total.
"""

import numpy as np

import concourse.bass as bass
import concourse.tile as tile
from concourse import bacc
from concourse import mybir
from concourse import bass_utils

F32 = mybir.dt.float32
F32R = mybir.dt.float32r
EXP = mybir.ActivationFunctionType.Exp

B, S, D, H = 2, 2048, 1024, 16
HG = 4                  # heads per core
E = 64                  # head dim
DG = HG * E             # 256, d-slice per core
NC = 8                  # cores
IT = 512                # query block (moving dim of attention matmuls)
JT = 128                # key tile
KC = D // 128           # 8 contraction chunks for projections
NSC = S // IT           # 4 s-chunks of 512
SCALE = 1.0 / np.sqrt(E)
SK = 2                  # ctx-matmul skew (att pipeline depth, in units)

_CACHE = {}


def _build():
    nc = bacc.Bacc("TRN2", target_bir_lowering=False, debug=False)

    xp = nc.dram_tensor("xp", [128, NSC * KC * IT], F32R, kind="ExternalInput").ap()
    wq = nc.dram_tensor("wq", [128, KC * DG], F32R, kind="ExternalInput").ap()
    wk = nc.dram_tensor("wk", [128, KC * DG], F32R, kind="ExternalInput").ap()
    wv = nc.dram_tensor("wv", [128, KC * DG], F32R, kind="ExternalInput").ap()
    wo = nc.dram_tensor("wo", [128, 2 * D], F32R, kind="ExternalInput").ap()
    on = nc.dram_tensor("on", [128, 128], F32R, kind="ExternalInput").ap()
    out = nc.dram_tensor("out", [S, D], F32, kind="ExternalOutput").ap()

    with tile.TileContext(nc) as tc:
        from contextlib import ExitStack

        with ExitStack() as ctx:
            pers = ctx.enter_context(tc.tile_pool(name="pers", bufs=1))

            wq_sb = pers.tile([128, KC * DG], F32R, tag="wq")         # 8 KB/p
            wk_sb = pers.tile([128, KC * DG], F32R, tag="wk")
            wv_sb = pers.tile([128, KC * DG], F32R, tag="wv")
            wo_sb = pers.tile([128, 2 * D], F32R, tag="wo")
            qT_sb = pers.tile([128, 2 * S], F32R, tag="qT")           # 16 KB/p
            kT_sb = pers.tile([128, 2 * S], F32R, tag="kT")
            v_sb = pers.tile([128, 16 * HG * (E + 1)], F32R, tag="v")  # 16.25 KB/p
            cx_sb = pers.tile([128, 2 * S], F32R, tag="cx")

            v4 = v_sb.rearrange("p (t h x) -> p t h x", t=16, h=HG)
            VW = HG * (E + 1)  # 260

            xs_pool = ctx.enter_context(tc.tile_pool(name="xsp", bufs=3))
            pt_pool = ctx.enter_context(tc.tile_pool(name="ptp", bufs=3))
            ot_pool = ctx.enter_context(tc.tile_pool(name="otp", bufs=2))
            dn_pool = ctx.enter_context(tc.tile_pool(name="dnp", bufs=2))
            sp_pool = ctx.enter_context(tc.tile_pool(name="spp", bufs=2, space="PSUM"))
            cp_pool = ctx.enter_context(tc.tile_pool(name="cpp", bufs=2, space="PSUM"))
            pj_pool = ctx.enter_context(tc.tile_pool(name="pjp", bufs=1, space="PSUM"))
            op_pool = ctx.enter_context(tc.tile_pool(name="opp", bufs=1, space="PSUM"))

            zero_reg = nc.gpsimd.to_reg(0.0)

            # ---- prologue DMAs (all contiguous; first chunks split out so
            # the first matmul can start after ~2 small transfers) ----
            nc.sync.dma_start(wq_sb[:, 0:DG], wq[:, 0:DG])
            nc.sync.dma_start(wk_sb[:, 0:DG], wk[:, 0:DG])
            xs_tiles = []
            xs0 = xs_pool.tile([128, KC, IT], F32R, tag="xs", name="xs0")
            xs_tiles.append(xs0)
            for k in range(KC):
                nc.sync.dma_start(xs0[:, k, :], xp[:, k * IT:(k + 1) * IT])
            nc.sync.dma_start(wq_sb[:, DG:], wq[:, DG:])
            nc.sync.dma_start(wk_sb[:, DG:], wk[:, DG:])
            nc.sync.dma_start(wv_sb[:], wv[:])
            xs1 = xs_pool.tile([128, KC, IT], F32R, tag="xs", name="xs1")
            xs_tiles.append(xs1)
            nc.sync.dma_start(xs1[:], xp[:, KC * IT:2 * KC * IT])
            nc.sync.dma_start(wo_sb[:], wo[:])
            # ones column of v (softmax denominator rides the ctx matmul)
            ones_sb = pers.tile([128, 128], F32R, tag="ones")
            nc.sync.dma_start(ones_sb[:], on[:])
            nc.vector.tensor_copy(
                v4[:, :, :, E], ones_sb[:, 0:64].rearrange("p (t h) -> p t h", h=HG))

            # ---- attention pipeline state machine ----
            ctx_q = []        # (emit_fn, group_end_fn | None)
            op_q = []         # pending out-proj emitters (prev query block)
            norms_open = [0]  # cx writes not yet emitted

            def tick():
                if op_q and norms_open[0] == 0:
                    op_q.pop(0)()

            def drain_ctx():
                emit, group_end = ctx_q.pop(0)
                emit()
                if group_end is not None:
                    group_end()

            cps_cur = {}      # head e -> cps tile for the pair in flight

            def emit_norm(cps, p, e, ti):
                # den row (part 64 of cps) -> K=1 PE broadcast matmul into the
                # proj psum slot -> recip -> scale ctx. Emitted inline at
                # group-end so the cps bank's next writer (the following
                # pair's first ctx matmul, drained one unit later) is emitted
                # after this read. (gpsimd.partition_broadcast would force a
                # GPSIMD microcode library swap against affine_select: ~8us.)
                dnr = dn_pool.tile([1, IT], F32R, tag="dnr")
                nc.vector.tensor_copy(dnr[:, :], cps[E:E + 1, :])
                dnb = op_pool.tile([128, IT], F32, name="dnb", tag="ops")
                nc.tensor.matmul(dnb[:], lhsT=ones_sb[0:1, :], rhs=dnr[:, :],
                                 start=True, stop=True)
                rc = dn_pool.tile([64, IT], F32, tag="rc")
                nc.vector.reciprocal_approx_fast(rc[:, :], dnb[0:64, :])
                nc.vector.tensor_mul(
                    cx_sb[e * E:(e + 1) * E, p * S + ti * IT: p * S + (ti + 1) * IT],
                    cps[0:E, :], rc[:, :])
                norms_open[0] -= 1

            def att_unit(ti, p, jj, njt):
                def go():
                    d = jj - 4 * ti
                    o = min(max(d, 0) * JT, 256)
                    n = IT - o
                    sp = sp_pool.tile([128, 2, IT], F32, tag="sp")
                    for e in range(2):
                        # row-tiled K=64 pair: e=0 rows 0:64 / e=1 rows 64:128
                        # of the PE array run concurrently
                        nc.tensor.matmul(
                            sp[:, e, o:IT],
                            lhsT=kT_sb[e * E:(e + 1) * E, p * S + jj * JT: p * S + jj * JT + JT],
                            rhs=qT_sb[e * E:(e + 1) * E, p * S + ti * IT + o: p * S + (ti + 1) * IT],
                            start=True, stop=True,
                        )
                    pt = pt_pool.tile([128, 2, IT], F32R, tag="pt")
                    nc.scalar.activation(pt[:, :, o:IT], sp[:, :, o:IT], EXP, scale=SCALE)
                    if d >= 0:
                        # keep where query_pos >= key_pos
                        nc.gpsimd.affine_select(
                            pt[:, :, o:IT], pt[:, :, o:IT],
                            pattern=[[0, 2], [1, n]],
                            compare_op=mybir.AluOpType.is_ge,
                            fill=zero_reg,
                            base=ti * IT + o - jj * JT,
                            channel_multiplier=-1,
                        )

                    def emit_ctx(pt=pt, o=o, jj=jj, njt=njt):
                        if jj == 0:
                            for e in range(2):
                                cps_cur[e] = cp_pool.tile([128, IT], F32,
                                                          name="cps", tag="cps")
                        for e in range(2):
                            nc.tensor.matmul(
                                cps_cur[e][0:E + 1, o:IT],
                                lhsT=v_sb[:, jj * VW + (2 * p + e) * (E + 1):
                                          jj * VW + (2 * p + e + 1) * (E + 1)],
                                rhs=pt[:, e, o:IT],
                                start=(jj == 0), stop=(jj == njt - 1),
                            )
                    group_end = None
                    if jj == njt - 1:
                        norms_open[0] += 2
                        def group_end(p=p, ti=ti):
                            for e in range(2):
                                emit_norm(cps_cur[e], p, e, ti)
                    ctx_q.append((emit_ctx, group_end))
                    if len(ctx_q) > SK:
                        drain_ctx()
                    tick()
                return go

            def emit_op(ti):
                # for the final block the proj pool is idle: alternate into it
                # so out-proj matmuls overlap the staging copies
                def go_all():
                    for it_ in range(4 * ti, 4 * ti + 4):
                        def go(it_=it_):
                            ot = ot_pool.tile([128, 2 * IT], F32, tag="ott")
                            for dc in range(2):
                                pool = pj_pool if (ti == NSC - 1 and dc == 1) else op_pool
                                tag = "pj" if (ti == NSC - 1 and dc == 1) else "ops"
                                ps = pool.tile([128, IT], F32, name="ops", tag=tag)
                                for pr in range(2):
                                    nc.tensor.matmul(
                                        ps[:],
                                        lhsT=cx_sb[:, pr * S + it_ * JT: pr * S + it_ * JT + JT],
                                        rhs=wo_sb[:, pr * D + dc * IT: pr * D + (dc + 1) * IT],
                                        start=(pr == 0), stop=(pr == 1),
                                    )
                                nc.vector.tensor_copy(ot[:, dc * IT:(dc + 1) * IT], ps[:])
                            nc.sync.dma_start(out[it_ * JT:(it_ + 1) * JT, :], ot[:])
                        op_q.append(go)
                return go_all

            # ---- projection emitters ----
            def qk_group(sc, pair, which):
                def go():
                    ps = pj_pool.tile([128, IT], F32, name="pjt", tag="pj")
                    w_sb = wq_sb if which == "q" else wk_sb
                    for k in range(KC):
                        nc.tensor.matmul(
                            ps[:],
                            lhsT=w_sb[:, k * DG + pair * 128: k * DG + (pair + 1) * 128],
                            rhs=xs_tiles[sc][:, k, :],
                            start=(k == 0), stop=(k == KC - 1),
                        )
                    dst = qT_sb if which == "q" else kT_sb
                    nc.vector.tensor_copy(
                        dst[:, pair * S + sc * IT: pair * S + (sc + 1) * IT], ps[:])
                return go

            def v_group(sc, st):
                def go():
                    ps = pj_pool.tile([128, IT], F32, name="pjt", tag="pj")
                    for k in range(KC):
                        nc.tensor.matmul(
                            ps[:, 0:DG],
                            lhsT=xs_tiles[sc][:, k, st * JT:(st + 1) * JT],
                            rhs=wv_sb[:, k * DG:(k + 1) * DG],
                            start=(k == 0), stop=(k == KC - 1),
                        )
                    nc.vector.tensor_copy(
                        v4[:, sc * 4 + st, :, 0:E],
                        ps[:, 0:DG].rearrange("p (h e) -> p h e", e=E))
                return go

            # ---- phase driver ----
            # Phase ti: projections for chunk ti AND attention for query
            # block ti (its off-diagonal units only need previous chunks'
            # k/v; diagonal units follow this chunk's k/v groups). Every
            # phase, including the last, has proj matmuls to fill the PE
            # while ACT chews on exp supertiles.
            def merge(groups, units, gcyc, ucyc):
                tp, tu = len(groups) * gcyc, len(units) * ucyc
                pc = uc = 0
                while groups or units:
                    if groups and (not units or pc * tu <= uc * tp):
                        groups.pop(0)()
                        pc += gcyc
                    else:
                        units.pop(0)()
                        uc += ucyc

            for ti in range(NSC):
                sc = ti
                if sc + 2 < NSC:
                    xs_n = xs_pool.tile([128, KC, IT], F32R, tag="xs",
                                        name=f"xs{sc + 2}")
                    xs_tiles.append(xs_n)
                    nc.sync.dma_start(
                        xs_n[:], xp[:, (sc + 2) * KC * IT:(sc + 3) * KC * IT])
                njt = 4 * (ti + 1)
                qk_group(sc, 0, "q")()
                qk_group(sc, 0, "k")()
                # pair0 off-diagonal units interleaved with remaining groups
                rest = [qk_group(sc, 1, "q"), qk_group(sc, 1, "k"),
                        v_group(sc, 0), v_group(sc, 1),
                        v_group(sc, 2), v_group(sc, 3)]
                p0_off = [att_unit(ti, 0, jj, njt) for jj in range(4 * ti)]
                merge(rest, p0_off, 3072, 1536)
                for jj in range(4 * ti, njt):          # pair0 diagonal
                    att_unit(ti, 0, jj, njt)()
                for jj in range(njt):                  # pair1
                    att_unit(ti, 1, jj, njt)()
                emit_op(ti)()

            while ctx_q:
                drain_ctx()
                tick()
            for _ in range(80):
                if not op_q:
                    break
                tick()
            assert not op_q and norms_open[0] == 0

    nc.compile()
    return nc


def _pack_x(xb):
    # x[b] [2048, 1024] -> [128, 4*8*512]: chunk (sc, k) = xT[k*128:+128, sc*512:+512]
    return np.ascontiguousarray(
        xb.reshape(NSC, IT, KC, 128).transpose(3, 0, 2, 1).reshape(128, NSC * KC * IT))


def _pack_w(w):
    # [1024, 256] -> [128, 8*256] chunk-major
    return np.ascontiguousarray(
        w.reshape(KC, 128, DG).transpose(1, 0, 2).reshape(128, KC * DG))


def _pack_wo(w):
    # [256, 1024] -> [128, 2*1024] pair-major
    return np.ascontiguousarray(
        w.reshape(2, 128, D).transpose(1, 0, 2).reshape(128, 2 * D))


def _in_maps(x, Wq, Wk, Wv, Wo):
    maps = []
    for c in range(NC):
        b, g = c // (NC // B), c % (NC // B)
        maps.append({
            "xp": _pack_x(x[b]),
            "wq": _pack_w(Wq[:, g * DG:(g + 1) * DG]),
            "wk": _pack_w(Wk[:, g * DG:(g + 1) * DG]),
            "wv": _pack_w(Wv[:, g * DG:(g + 1) * DG]),
            "wo": _pack_wo(Wo[g * DG:(g + 1) * DG, :]),
            "on": np.ones((128, 128), np.float32),
        })
    return maps


def run(x, Wq, Wk, Wv, Wo, bo, trace=False):
    if "nc" not in _CACHE:
        _CACHE["nc"] = _build()
    nc = _CACHE["nc"]
    res = bass_utils.run_bass_kernel_spmd(
        nc, _in_maps(x, Wq, Wk, Wv, Wo), core_ids=list(range(NC)), trace=trace,
    )
    parts = [res.results[c]["out"] for c in range(NC)]
    gpb = NC // B
    full = np.stack([sum(parts[b * gpb + 1: (b + 1) * gpb], parts[b * gpb]) for b in range(B)])
    full = full + np.asarray(bo, np.float32)[None, None, :]
    return full.astype(np.float32), res


def kernel(x, Wq, Wk, Wv, Wo, bo):
    x = np.asarray(x, np.float32)
    full, _ = run(x, np.asarray(Wq, np.float32), np.asarray(Wk, np.float32),
                  np.asarray(Wv, np.float32), np.asarray(Wo, np.float32),
                  np.asarray(bo, np.float32))
    return full


# revision 22
# speedup vs baseline: 1.0897x; 1.0564x over previous
"""Multi-head causal attention (b=2, s=2048, d=1024, h=16) on 8 TRN2 cores.

Sharding: batch (2) x head-groups (4 heads each) -> 8 cores, Megatron-style.
Each core: QKV col-sliced projections (d -> 256), causal attention for its 4
heads, row-sliced output projection producing a partial [2048, 1024] output.
Host sums the 4 partials per batch and adds the output bias.

v2 design (fused streaming pipeline):
  - One pass over s in 4 chunks of 512: projections for chunk sc are emitted
    interleaved with attention for query block sc-1, so PE, ACT (exp), DVE
    and GPSIMD stay busy end-to-end and the PE HAM clock never re-throttles.
  - Heads processed in PAIRS stacked on partition halves: q/k projections are
    unpadded [d,128] blocks (half the baseline's PE work), scores for the two
    heads of a pair run CONCURRENTLY as row-tiled K=64 matmuls (tile_position
    (0,0)/(64,0)), halving effective scores time.
  - The two concurrent score matmuls write one [128, 2, 512] PSUM supertile
    (2 banks) so a single ACT exp instruction covers both heads, halving the
    per-instruction ACT overhead (352 cycles/instr).
  - Causal mask via gpsimd.affine_select on the exp output (no tri tensor,
    no mask DMA); diagonal o-offsets clamped to <=256 so every fp32r matmul
    keeps moving dim >= 256 (below that fp32r runs 4x slower).
  - Softmax denominator from an extra ones-column in v (row 64 of ctx PSUM),
    broadcast via gpsimd.partition_broadcast (no PE broadcast matmul).
  - All inputs host-packed so every DMA is a contiguous per-partition blob:
    4 weight DMAs + 11 x DMAs + 32 output DMAs total.
"""

import numpy as np

import concourse.bass as bass
import concourse.tile as tile
from concourse import bacc
from concourse import mybir
from concourse import bass_utils

F32 = mybir.dt.float32
F32R = mybir.dt.float32r
EXP = mybir.ActivationFunctionType.Exp

B, S, D, H = 2, 2048, 1024, 16
HG = 4                  # heads per core
E = 64                  # head dim
DG = HG * E             # 256, d-slice per core
NC = 8                  # cores
IT = 512                # query block (moving dim of attention matmuls)
JT = 128                # key tile
KC = D // 128           # 8 contraction chunks for projections
NSC = S // IT           # 4 s-chunks of 512
SCALE = 1.0 / np.sqrt(E)
SK = 3                  # ctx-matmul skew (att pipeline depth, in units)

_CACHE = {}


def _build():
    nc = bacc.Bacc("TRN2", target_bir_lowering=False, debug=False)

    xp = nc.dram_tensor("xp", [128, NSC * KC * IT], F32R, kind="ExternalInput").ap()
    wq = nc.dram_tensor("wq", [128, KC * DG], F32R, kind="ExternalInput").ap()
    wk = nc.dram_tensor("wk", [128, KC * DG], F32R, kind="ExternalInput").ap()
    wv = nc.dram_tensor("wv", [128, KC * DG], F32R, kind="ExternalInput").ap()
    wo = nc.dram_tensor("wo", [128, 2 * D], F32R, kind="ExternalInput").ap()
    wf = nc.dram_tensor("wf", [128, 2 * DG + 128], F32R, kind="ExternalInput").ap()
    out = nc.dram_tensor("out", [S, D], F32, kind="ExternalOutput").ap()

    with tile.TileContext(nc) as tc:
        from contextlib import ExitStack

        with ExitStack() as ctx:
            pers = ctx.enter_context(tc.tile_pool(name="pers", bufs=1))

            wq_sb = pers.tile([128, KC * DG], F32R, tag="wq")         # 8 KB/p
            wk_sb = pers.tile([128, KC * DG], F32R, tag="wk")
            wv_sb = pers.tile([128, KC * DG], F32R, tag="wv")
            wo_sb = pers.tile([128, 2 * D], F32R, tag="wo")
            qT_sb = pers.tile([128, 2 * S], F32R, tag="qT")           # 16 KB/p
            kT_sb = pers.tile([128, 2 * S], F32R, tag="kT")
            v_sb = pers.tile([128, 16 * HG * (E + 1)], F32R, tag="v")  # 16.25 KB/p
            cx_sb = pers.tile([128, 2 * S], F32R, tag="cx")

            v4 = v_sb.rearrange("p (t h x) -> p t h x", t=16, h=HG)
            VW = HG * (E + 1)  # 260

            xs_pool = ctx.enter_context(tc.tile_pool(name="xsp", bufs=3))
            pt_pool = ctx.enter_context(tc.tile_pool(name="ptp", bufs=5))
            ot_pool = ctx.enter_context(tc.tile_pool(name="otp", bufs=2))
            dn_pool = ctx.enter_context(tc.tile_pool(name="dnp", bufs=2))
            sp_pool = ctx.enter_context(tc.tile_pool(name="spp", bufs=2, space="PSUM"))
            cp_pool = ctx.enter_context(tc.tile_pool(name="cpp", bufs=2, space="PSUM"))
            pj_pool = ctx.enter_context(tc.tile_pool(name="pjp", bufs=1, space="PSUM"))
            op_pool = ctx.enter_context(tc.tile_pool(name="opp", bufs=1, space="PSUM"))

            zero_reg = nc.gpsimd.to_reg(0.0)

            # ---- prologue DMAs: first q-matmul needs only wf's first
            # column block + x chunk 0; everything else streams behind ----
            ones_sb = pers.tile([128, 128], F32R, tag="ones")
            xs_tiles = []
            xs0 = xs_pool.tile([128, KC, IT], F32R, tag="xs", name="xs0")
            xs_tiles.append(xs0)
            nc.sync.dma_start(wq_sb[:, 0:DG], wf[:, 0:DG])
            nc.sync.dma_start(xs0[:, 0, :], xp[:, 0:IT])
            nc.sync.dma_start(wk_sb[:, 0:DG], wf[:, DG:2 * DG])
            nc.sync.dma_start(xs0[:, 1, :], xp[:, IT:2 * IT])
            nc.sync.dma_start(wq_sb[:, DG:], wq[:, DG:])
            nc.sync.dma_start(xs0[:, 2, :], xp[:, 2 * IT:3 * IT])
            nc.sync.dma_start(wk_sb[:, DG:], wk[:, DG:])
            for k in range(3, KC):
                nc.sync.dma_start(xs0[:, k, :], xp[:, k * IT:(k + 1) * IT])
            nc.sync.dma_start(ones_sb[:], wf[:, 2 * DG:])
            nc.sync.dma_start(wv_sb[:], wv[:])
            xs1 = xs_pool.tile([128, KC, IT], F32R, tag="xs", name="xs1")
            xs_tiles.append(xs1)
            nc.sync.dma_start(xs1[:], xp[:, KC * IT:2 * KC * IT])
            nc.sync.dma_start(wo_sb[:], wo[:])
            # ones column of v (softmax denominator rides the ctx matmul)
            nc.vector.tensor_copy(
                v4[:, :, :, E], ones_sb[:, 0:64].rearrange("p (t h) -> p t h", h=HG))

            # ---- attention pipeline state machine ----
            ctx_q = []        # (emit_fn, group_end_fn | None)
            op_q = []         # pending out-proj emitters (prev query block)
            norms_open = [0]  # cx writes not yet emitted
            op_gate = [True]  # hold out-proj pops for the pair1 stretch

            def tick():
                if op_q and norms_open[0] == 0 and op_gate[0]:
                    op_q.pop(0)()

            def drain_ctx():
                emit, group_end = ctx_q.pop(0)
                emit()
                if group_end is not None:
                    group_end()

            cps_cur = {}      # head e -> cps tile for the pair in flight

            def emit_norm(cps, p, e, ti):
                # den row (part 64 of cps) -> K=1 PE broadcast matmul into the
                # proj psum slot -> recip -> scale ctx. Emitted inline at
                # group-end so the cps bank's next writer (the following
                # pair's first ctx matmul, drained one unit later) is emitted
                # after this read. (gpsimd.partition_broadcast would force a
                # GPSIMD microcode library swap against affine_select: ~8us.)
                dnr = dn_pool.tile([1, IT], F32R, tag="dnr")
                nc.vector.tensor_copy(dnr[:, :], cps[E:E + 1, :])
                dnb = op_pool.tile([128, IT], F32, name="dnb", tag="ops")
                nc.tensor.matmul(dnb[:], lhsT=ones_sb[0:1, :], rhs=dnr[:, :],
                                 start=True, stop=True)
                rc = dn_pool.tile([64, IT], F32, tag="rc")
                nc.vector.reciprocal_approx_fast(rc[:, :], dnb[0:64, :])
                nc.vector.tensor_mul(
                    cx_sb[e * E:(e + 1) * E, p * S + ti * IT: p * S + (ti + 1) * IT],
                    cps[0:E, :], rc[:, :])
                norms_open[0] -= 1

            def att_unit(ti, p, jj, njt):
                def go():
                    d = jj - 4 * ti
                    o = min(max(d, 0) * JT, 256)
                    n = IT - o
                    sp = sp_pool.tile([128, 2, IT], F32, tag="sp")
                    for e in range(2):
                        # row-tiled K=64 pair: e=0 rows 0:64 / e=1 rows 64:128
                        # of the PE array run concurrently
                        nc.tensor.matmul(
                            sp[:, e, o:IT],
                            lhsT=kT_sb[e * E:(e + 1) * E, p * S + jj * JT: p * S + jj * JT + JT],
                            rhs=qT_sb[e * E:(e + 1) * E, p * S + ti * IT + o: p * S + (ti + 1) * IT],
                            start=True, stop=True,
                        )
                    pt = pt_pool.tile([128, 2, IT], F32R, tag="pt")
                    nc.scalar.activation(pt[:, :, o:IT], sp[:, :, o:IT], EXP, scale=SCALE)
                    if d >= 0:
                        # keep where query_pos >= key_pos
                        nc.gpsimd.affine_select(
                            pt[:, :, o:IT], pt[:, :, o:IT],
                            pattern=[[0, 2], [1, n]],
                            compare_op=mybir.AluOpType.is_ge,
                            fill=zero_reg,
                            base=ti * IT + o - jj * JT,
                            channel_multiplier=-1,
                        )

                    def emit_ctx(pt=pt, o=o, jj=jj, njt=njt):
                        if jj == 0:
                            for e in range(2):
                                cps_cur[e] = cp_pool.tile([128, IT], F32,
                                                          name="cps", tag="cps")
                        for e in range(2):
                            nc.tensor.matmul(
                                cps_cur[e][0:E + 1, o:IT],
                                lhsT=v_sb[:, jj * VW + (2 * p + e) * (E + 1):
                                          jj * VW + (2 * p + e + 1) * (E + 1)],
                                rhs=pt[:, e, o:IT],
                                start=(jj == 0), stop=(jj == njt - 1),
                            )
                    group_end = None
                    if jj == njt - 1:
                        norms_open[0] += 2
                        def group_end(p=p, ti=ti):
                            for e in range(2):
                                emit_norm(cps_cur[e], p, e, ti)
                    ctx_q.append((emit_ctx, group_end))
                    if len(ctx_q) > SK:
                        drain_ctx()
                    tick()
                return go

            def emit_op(ti):
                # for the final block the proj pool is idle: alternate into it
                # so out-proj matmuls overlap the staging copies
                def go_all():
                    for it_ in range(4 * ti, 4 * ti + 4):
                        def go(it_=it_):
                            ot = ot_pool.tile([128, 2 * IT], F32, tag="ott")
                            for dc in range(2):
                                pool = pj_pool if (ti == NSC - 1 and dc == 1) else op_pool
                                tag = "pj" if (ti == NSC - 1 and dc == 1) else "ops"
                                ps = pool.tile([128, IT], F32, name="ops", tag=tag)
                                for pr in range(2):
                                    nc.tensor.matmul(
                                        ps[:],
                                        lhsT=cx_sb[:, pr * S + it_ * JT: pr * S + it_ * JT + JT],
                                        rhs=wo_sb[:, pr * D + dc * IT: pr * D + (dc + 1) * IT],
                                        start=(pr == 0), stop=(pr == 1),
                                    )
                                nc.vector.tensor_copy(ot[:, dc * IT:(dc + 1) * IT], ps[:])
                                nc.sync.dma_start(
                                    out[it_ * JT:(it_ + 1) * JT, dc * IT:(dc + 1) * IT],
                                    ot[:, dc * IT:(dc + 1) * IT])
                        op_q.append(go)
                return go_all

            # ---- projection emitters ----
            def qk_group(sc, pair, which):
                def go():
                    ps = pj_pool.tile([128, IT], F32, name="pjt", tag="pj")
                    w_sb = wq_sb if which == "q" else wk_sb
                    for k in range(KC):
                        nc.tensor.matmul(
                            ps[:],
                            lhsT=w_sb[:, k * DG + pair * 128: k * DG + (pair + 1) * 128],
                            rhs=xs_tiles[sc][:, k, :],
                            start=(k == 0), stop=(k == KC - 1),
                        )
                    dst = qT_sb if which == "q" else kT_sb
                    nc.vector.tensor_copy(
                        dst[:, pair * S + sc * IT: pair * S + (sc + 1) * IT], ps[:])
                return go

            def v_group(sc, st):
                def go():
                    ps = pj_pool.tile([128, IT], F32, name="pjt", tag="pj")
                    for k in range(KC):
                        nc.tensor.matmul(
                            ps[:, 0:DG],
                            lhsT=xs_tiles[sc][:, k, st * JT:(st + 1) * JT],
                            rhs=wv_sb[:, k * DG:(k + 1) * DG],
                            start=(k == 0), stop=(k == KC - 1),
                        )
                    nc.vector.tensor_copy(
                        v4[:, sc * 4 + st, :, 0:E],
                        ps[:, 0:DG].rearrange("p (h e) -> p h e", e=E))
                return go

            # ---- phase driver ----
            # Phase ti: projections for chunk ti AND attention for query
            # block ti (its off-diagonal units only need previous chunks'
            # k/v; diagonal units follow this chunk's k/v groups). Every
            # phase, including the last, has proj matmuls to fill the PE
            # while ACT chews on exp supertiles.
            def merge(groups, units, gcyc, ucyc):
                tp, tu = len(groups) * gcyc, len(units) * ucyc
                pc = uc = 0
                while groups or units:
                    if groups and (not units or pc * tu <= uc * tp):
                        groups.pop(0)()
                        pc += gcyc
                    else:
                        units.pop(0)()
                        uc += ucyc

            for ti in range(NSC):
                sc = ti
                if sc + 2 < NSC:
                    xs_n = xs_pool.tile([128, KC, IT], F32R, tag="xs",
                                        name=f"xs{sc + 2}")
                    xs_tiles.append(xs_n)
                    nc.sync.dma_start(
                        xs_n[:], xp[:, (sc + 2) * KC * IT:(sc + 3) * KC * IT])
                njt = 4 * (ti + 1)
                op_gate[0] = False
                qk_group(sc, 0, "q")()
                qk_group(sc, 0, "k")()
                # pair0 off-diagonal units interleaved with remaining groups
                rest = [qk_group(sc, 1, "q"), qk_group(sc, 1, "k"),
                        v_group(sc, 0), v_group(sc, 1),
                        v_group(sc, 2), v_group(sc, 3)]
                p0_off = [att_unit(ti, 0, jj, njt) for jj in range(4 * ti)]
                merge(rest, p0_off, 3072, 1536)
                op_gate[0] = True   # out-proj fills the group-free stretch
                for jj in range(4 * ti, njt):          # pair0 diagonal
                    att_unit(ti, 0, jj, njt)()
                for jj in range(njt):                  # pair1
                    att_unit(ti, 1, jj, njt)()
                emit_op(ti)()

            while ctx_q:
                drain_ctx()
                tick()
            for _ in range(80):
                if not op_q:
                    break
                tick()
            assert not op_q and norms_open[0] == 0

    nc.compile()
    return nc


def _pack_x(xb):
    # x[b] [2048, 1024] -> [128, 4*8*512]: chunk (sc, k) = xT[k*128:+128, sc*512:+512]
    return np.ascontiguousarray(
        xb.reshape(NSC, IT, KC, 128).transpose(3, 0, 2, 1).reshape(128, NSC * KC * IT))


def _pack_w(w):
    # [1024, 256] -> [128, 8*256] chunk-major
    return np.ascontiguousarray(
        w.reshape(KC, 128, DG).transpose(1, 0, 2).reshape(128, KC * DG))


def _pack_wo(w):
    # [256, 1024] -> [128, 2*1024] pair-major
    return np.ascontiguousarray(
        w.reshape(2, 128, D).transpose(1, 0, 2).reshape(128, 2 * D))


def _in_maps(x, Wq, Wk, Wv, Wo):
    maps = []
    ones = np.ones((128, 128), np.float32)
    for c in range(NC):
        b, g = c // (NC // B), c % (NC // B)
        wqp = _pack_w(Wq[:, g * DG:(g + 1) * DG])
        wkp = _pack_w(Wk[:, g * DG:(g + 1) * DG])
        maps.append({
            "xp": _pack_x(x[b]),
            "wq": wqp,
            "wk": wkp,
            "wv": _pack_w(Wv[:, g * DG:(g + 1) * DG]),
            "wo": _pack_wo(Wo[g * DG:(g + 1) * DG, :]),
            "wf": np.ascontiguousarray(
                np.concatenate([wqp[:, 0:DG], wkp[:, 0:DG], ones], axis=1)),
        })
    return maps


def run(x, Wq, Wk, Wv, Wo, bo, trace=False):
    if "nc" not in _CACHE:
        _CACHE["nc"] = _build()
    nc = _CACHE["nc"]
    res = bass_utils.run_bass_kernel_spmd(
        nc, _in_maps(x, Wq, Wk, Wv, Wo), core_ids=list(range(NC)), trace=trace,
    )
    parts = [res.results[c]["out"] for c in range(NC)]
    gpb = NC // B
    full = np.stack([sum(parts[b * gpb + 1: (b + 1) * gpb], parts[b * gpb]) for b in range(B)])
    full = full + np.asarray(bo, np.float32)[None, None, :]
    return full.astype(np.float32), res


def kernel(x, Wq, Wk, Wv, Wo, bo):
    x = np.asarray(x, np.float32)
    full, _ = run(x, np.asarray(Wq, np.float32), np.asarray(Wk, np.float32),
                  np.asarray(Wv, np.float32), np.asarray(Wo, np.float32),
                  np.asarray(bo, np.float32))
    return full


# revision 24
# speedup vs baseline: 1.2592x; 1.1555x over previous
"""Multi-head causal attention (b=2, s=2048, d=1024, h=16) on 8 TRN2 cores.

Sharding: batch (2) x head-groups (4 heads each) -> 8 cores, Megatron-style.
Each core: QKV col-sliced projections (d -> 256), causal attention for its 4
heads, row-sliced output projection producing a partial [2048, 1024] output.
Host sums the 4 partials per batch and adds the output bias.

v2 design (fused streaming pipeline):
  - One pass over s in 4 chunks of 512: projections for chunk sc are emitted
    interleaved with attention for query block sc-1, so PE, ACT (exp), DVE
    and GPSIMD stay busy end-to-end and the PE HAM clock never re-throttles.
  - Heads processed in PAIRS stacked on partition halves: q/k projections are
    unpadded [d,128] blocks (half the baseline's PE work), scores for the two
    heads of a pair run CONCURRENTLY as row-tiled K=64 matmuls (tile_position
    (0,0)/(64,0)), halving effective scores time.
  - The two concurrent score matmuls write one [128, 2, 512] PSUM supertile
    (2 banks) so a single ACT exp instruction covers both heads, halving the
    per-instruction ACT overhead (352 cycles/instr).
  - Causal mask via gpsimd.affine_select on the exp output (no tri tensor,
    no mask DMA); diagonal o-offsets clamped to <=256 so every fp32r matmul
    keeps moving dim >= 256 (below that fp32r runs 4x slower).
  - Softmax denominator from an extra ones-column in v (row 64 of ctx PSUM),
    broadcast via gpsimd.partition_broadcast (no PE broadcast matmul).
  - All inputs host-packed so every DMA is a contiguous per-partition blob:
    4 weight DMAs + 11 x DMAs + 32 output DMAs total.
"""

import ml_dtypes
import numpy as np

import concourse.bass as bass
import concourse.tile as tile
from concourse import bacc
from concourse import mybir
from concourse import bass_utils

F32 = mybir.dt.float32
F32R = mybir.dt.float32r
BF16 = mybir.dt.bfloat16
EXP = mybir.ActivationFunctionType.Exp

B, S, D, H = 2, 2048, 1024, 16
HG = 4                  # heads per core
E = 64                  # head dim
DG = HG * E             # 256, d-slice per core
NC = 8                  # cores
IT = 512                # query block (moving dim of attention matmuls)
JT = 128                # key tile
KC = D // 128           # 8 contraction chunks for projections
NSC = S // IT           # 4 s-chunks of 512
SCALE = 1.0 / np.sqrt(E)
SK = 3                  # ctx-matmul skew (att pipeline depth, in units)

_CACHE = {}


def _build():
    nc = bacc.Bacc("TRN2", target_bir_lowering=False, debug=False)

    xp = nc.dram_tensor("xp", [128, NSC * KC * IT], BF16, kind="ExternalInput").ap()
    wq = nc.dram_tensor("wq", [128, KC * DG], BF16, kind="ExternalInput").ap()
    wk = nc.dram_tensor("wk", [128, KC * DG], BF16, kind="ExternalInput").ap()
    wv = nc.dram_tensor("wv", [128, KC * DG], BF16, kind="ExternalInput").ap()
    wo = nc.dram_tensor("wo", [128, 2 * D], BF16, kind="ExternalInput").ap()
    wf = nc.dram_tensor("wf", [128, 2 * DG + 128], BF16, kind="ExternalInput").ap()
    out = nc.dram_tensor("out", [S, D], F32, kind="ExternalOutput").ap()

    with tile.TileContext(nc) as tc:
        from contextlib import ExitStack

        with ExitStack() as ctx:
            pers = ctx.enter_context(tc.tile_pool(name="pers", bufs=1))

            wq_sb = pers.tile([128, KC * DG], BF16, tag="wq")         # 8 KB/p
            wk_sb = pers.tile([128, KC * DG], BF16, tag="wk")
            wv_sb = pers.tile([128, KC * DG], BF16, tag="wv")
            wo_sb = pers.tile([128, 2 * D], BF16, tag="wo")
            qT_sb = pers.tile([128, 2 * S], BF16, tag="qT")           # 16 KB/p
            kT_sb = pers.tile([128, 2 * S], BF16, tag="kT")
            v_sb = pers.tile([128, 16 * HG * (E + 1)], BF16, tag="v")  # 16.25 KB/p
            cx_sb = pers.tile([128, 2 * S], BF16, tag="cx")

            v4 = v_sb.rearrange("p (t h x) -> p t h x", t=16, h=HG)
            VW = HG * (E + 1)  # 260

            xs_pool = ctx.enter_context(tc.tile_pool(name="xsp", bufs=3))
            pt_pool = ctx.enter_context(tc.tile_pool(name="ptp", bufs=5))
            ot_pool = ctx.enter_context(tc.tile_pool(name="otp", bufs=3))
            dn_pool = ctx.enter_context(tc.tile_pool(name="dnp", bufs=2))
            sp_pool = ctx.enter_context(tc.tile_pool(name="spp", bufs=2, space="PSUM"))
            cp_pool = ctx.enter_context(tc.tile_pool(name="cpp", bufs=2, space="PSUM"))
            pj_pool = ctx.enter_context(tc.tile_pool(name="pjp", bufs=1, space="PSUM"))
            op_pool = ctx.enter_context(tc.tile_pool(name="opp", bufs=1, space="PSUM"))

            zero_reg = nc.gpsimd.to_reg(0.0)

            # ---- prologue DMAs: first q-matmul needs only wf's first
            # column block + x chunk 0; everything else streams behind ----
            ones_sb = pers.tile([128, 128], BF16, tag="ones")
            xs_tiles = []
            xs0 = xs_pool.tile([128, KC, IT], BF16, tag="xs", name="xs0")
            xs_tiles.append(xs0)
            nc.sync.dma_start(wq_sb[:, 0:DG], wf[:, 0:DG])
            nc.sync.dma_start(xs0[:, 0, :], xp[:, 0:IT])
            nc.sync.dma_start(wk_sb[:, 0:DG], wf[:, DG:2 * DG])
            nc.sync.dma_start(xs0[:, 1, :], xp[:, IT:2 * IT])
            nc.sync.dma_start(wq_sb[:, DG:], wq[:, DG:])
            nc.sync.dma_start(xs0[:, 2, :], xp[:, 2 * IT:3 * IT])
            nc.sync.dma_start(wk_sb[:, DG:], wk[:, DG:])
            for k in range(3, KC):
                nc.sync.dma_start(xs0[:, k, :], xp[:, k * IT:(k + 1) * IT])
            nc.sync.dma_start(ones_sb[:], wf[:, 2 * DG:])
            nc.sync.dma_start(wv_sb[:], wv[:])
            xs1 = xs_pool.tile([128, KC, IT], BF16, tag="xs", name="xs1")
            xs_tiles.append(xs1)
            nc.sync.dma_start(xs1[:], xp[:, KC * IT:2 * KC * IT])
            nc.sync.dma_start(wo_sb[:], wo[:])
            # ones column of v (softmax denominator rides the ctx matmul)
            nc.vector.tensor_copy(
                v4[:, :, :, E], ones_sb[:, 0:64].rearrange("p (t h) -> p t h", h=HG))

            # ---- attention pipeline state machine ----
            ctx_q = []        # (emit_fn, group_end_fn | None)
            op_q = []         # pending out-proj emitters (prev query block)
            norms_open = [0]  # cx writes not yet emitted
            op_gate = [True]  # hold out-proj pops for the pair1 stretch

            def tick():
                if op_q and norms_open[0] == 0 and op_gate[0]:
                    op_q.pop(0)()

            def drain_ctx():
                emit, group_end = ctx_q.pop(0)
                emit()
                if group_end is not None:
                    group_end()

            cps_cur = {}      # head e -> cps tile for the pair in flight

            def emit_norm(cps, p, e, ti):
                # den row (part 64 of cps) -> K=1 PE broadcast matmul into the
                # proj psum slot -> recip -> scale ctx. Emitted inline at
                # group-end so the cps bank's next writer (the following
                # pair's first ctx matmul, drained one unit later) is emitted
                # after this read. (gpsimd.partition_broadcast would force a
                # GPSIMD microcode library swap against affine_select: ~8us.)
                dnr = dn_pool.tile([1, IT], BF16, tag="dnr")
                nc.vector.tensor_copy(dnr[:, :], cps[E:E + 1, :])
                dnb = op_pool.tile([128, IT], F32, name="dnb", tag="ops")
                nc.tensor.matmul(dnb[:], lhsT=ones_sb[0:1, :], rhs=dnr[:, :],
                                 start=True, stop=True)
                rc = dn_pool.tile([64, IT], F32, tag="rc")
                nc.vector.reciprocal_approx_fast(rc[:, :], dnb[0:64, :])
                nc.vector.tensor_mul(
                    cx_sb[e * E:(e + 1) * E, p * S + ti * IT: p * S + (ti + 1) * IT],
                    cps[0:E, :], rc[:, :])
                norms_open[0] -= 1

            def att_unit(ti, p, jj, njt):
                def go():
                    d = jj - 4 * ti
                    o = min(max(d, 0) * JT, 256)
                    n = IT - o
                    sp = sp_pool.tile([128, 2, IT], F32, tag="sp")
                    for e in range(2):
                        # row-tiled K=64 pair: e=0 rows 0:64 / e=1 rows 64:128
                        # of the PE array run concurrently
                        nc.tensor.matmul(
                            sp[:, e, o:IT],
                            lhsT=kT_sb[e * E:(e + 1) * E, p * S + jj * JT: p * S + jj * JT + JT],
                            rhs=qT_sb[e * E:(e + 1) * E, p * S + ti * IT + o: p * S + (ti + 1) * IT],
                            start=True, stop=True,
                        )
                    pt = pt_pool.tile([128, 2, IT], BF16, tag="pt")
                    nc.scalar.activation(pt[:, :, o:IT], sp[:, :, o:IT], EXP, scale=SCALE)
                    if d >= 0:
                        # keep where query_pos >= key_pos
                        nc.gpsimd.affine_select(
                            pt[:, :, o:IT], pt[:, :, o:IT],
                            pattern=[[0, 2], [1, n]],
                            compare_op=mybir.AluOpType.is_ge,
                            fill=zero_reg,
                            base=ti * IT + o - jj * JT,
                            channel_multiplier=-1,
                        )

                    def emit_ctx(pt=pt, o=o, jj=jj, njt=njt):
                        if jj == 0:
                            for e in range(2):
                                cps_cur[e] = cp_pool.tile([128, IT], F32,
                                                          name="cps", tag="cps")
                        for e in range(2):
                            nc.tensor.matmul(
                                cps_cur[e][0:E + 1, o:IT],
                                lhsT=v_sb[:, jj * VW + (2 * p + e) * (E + 1):
                                          jj * VW + (2 * p + e + 1) * (E + 1)],
                                rhs=pt[:, e, o:IT],
                                start=(jj == 0), stop=(jj == njt - 1),
                            )
                    group_end = None
                    if jj == njt - 1:
                        norms_open[0] += 2
                        def group_end(p=p, ti=ti):
                            for e in range(2):
                                emit_norm(cps_cur[e], p, e, ti)
                    ctx_q.append((emit_ctx, group_end))
                    if len(ctx_q) > SK:
                        drain_ctx()
                    tick()
                return go

            def emit_op(ti):
                # for the final block the proj pool is idle: alternate into it
                # so out-proj matmuls overlap the staging copies
                def go_all():
                    for it_ in range(4 * ti, 4 * ti + 4):
                        def go(it_=it_):
                            ot = ot_pool.tile([128, 2 * IT], F32, tag="ott")
                            for dc in range(2):
                                pool = pj_pool if (ti == NSC - 1 and dc == 1) else op_pool
                                tag = "pj" if (ti == NSC - 1 and dc == 1) else "ops"
                                ps = pool.tile([128, IT], F32, name="ops", tag=tag)
                                for pr in range(2):
                                    nc.tensor.matmul(
                                        ps[:],
                                        lhsT=cx_sb[:, pr * S + it_ * JT: pr * S + it_ * JT + JT],
                                        rhs=wo_sb[:, pr * D + dc * IT: pr * D + (dc + 1) * IT],
                                        start=(pr == 0), stop=(pr == 1),
                                    )
                                nc.vector.tensor_copy(ot[:, dc * IT:(dc + 1) * IT], ps[:])
                                nc.sync.dma_start(
                                    out[it_ * JT:(it_ + 1) * JT, dc * IT:(dc + 1) * IT],
                                    ot[:, dc * IT:(dc + 1) * IT])
                        op_q.append(go)
                return go_all

            # ---- projection emitters ----
            def qk_group(sc, pair, which):
                def go():
                    ps = pj_pool.tile([128, IT], F32, name="pjt", tag="pj")
                    w_sb = wq_sb if which == "q" else wk_sb
                    for k in range(KC):
                        nc.tensor.matmul(
                            ps[:],
                            lhsT=w_sb[:, k * DG + pair * 128: k * DG + (pair + 1) * 128],
                            rhs=xs_tiles[sc][:, k, :],
                            start=(k == 0), stop=(k == KC - 1),
                        )
                    dst = qT_sb if which == "q" else kT_sb
                    # scalar engine: keeps the vector queue short so the
                    # latency-critical norm chain (which frees cps banks for
                    # the next pair's ctx matmuls) isn't stuck behind copies
                    nc.scalar.copy(
                        dst[:, pair * S + sc * IT: pair * S + (sc + 1) * IT], ps[:])
                return go

            def v_group(sc, st):
                def go():
                    ps = pj_pool.tile([128, IT], F32, name="pjt", tag="pj")
                    for k in range(KC):
                        nc.tensor.matmul(
                            ps[:, 0:DG],
                            lhsT=xs_tiles[sc][:, k, st * JT:(st + 1) * JT],
                            rhs=wv_sb[:, k * DG:(k + 1) * DG],
                            start=(k == 0), stop=(k == KC - 1),
                        )
                    nc.scalar.copy(
                        v4[:, sc * 4 + st, :, 0:E],
                        ps[:, 0:DG].rearrange("p (h e) -> p h e", e=E))
                return go

            # ---- phase driver ----
            # Phase ti: projections for chunk ti AND attention for query
            # block ti (its off-diagonal units only need previous chunks'
            # k/v; diagonal units follow this chunk's k/v groups). Every
            # phase, including the last, has proj matmuls to fill the PE
            # while ACT chews on exp supertiles.
            def merge(groups, units, gcyc, ucyc):
                tp, tu = len(groups) * gcyc, len(units) * ucyc
                pc = uc = 0
                while groups or units:
                    if groups and (not units or pc * tu <= uc * tp):
                        groups.pop(0)()
                        pc += gcyc
                    else:
                        units.pop(0)()
                        uc += ucyc

            for ti in range(NSC):
                sc = ti
                if sc + 2 < NSC:
                    xs_n = xs_pool.tile([128, KC, IT], BF16, tag="xs",
                                        name=f"xs{sc + 2}")
                    xs_tiles.append(xs_n)
                    nc.sync.dma_start(
                        xs_n[:], xp[:, (sc + 2) * KC * IT:(sc + 3) * KC * IT])
                njt = 4 * (ti + 1)
                op_gate[0] = False
                qk_group(sc, 0, "q")()
                qk_group(sc, 0, "k")()
                # pair0 off-diagonal units interleaved with remaining groups
                rest = [qk_group(sc, 1, "q"), qk_group(sc, 1, "k"),
                        v_group(sc, 0), v_group(sc, 1),
                        v_group(sc, 2), v_group(sc, 3)]
                p0_off = [att_unit(ti, 0, jj, njt) for jj in range(4 * ti)]
                merge(rest, p0_off, 3072, 1536)
                op_gate[0] = True   # out-proj fills the group-free stretch
                for jj in range(4 * ti, njt):          # pair0 diagonal
                    att_unit(ti, 0, jj, njt)()
                for jj in range(njt):                  # pair1
                    att_unit(ti, 1, jj, njt)()
                emit_op(ti)()

            while ctx_q:
                drain_ctx()
                tick()
            for _ in range(80):
                if not op_q:
                    break
                tick()
            assert not op_q and norms_open[0] == 0

    nc.compile()
    return nc


def _pack_x(xb):
    # x[b] [2048, 1024] -> [128, 4*8*512]: chunk (sc, k) = xT[k*128:+128, sc*512:+512]
    return np.ascontiguousarray(
        xb.reshape(NSC, IT, KC, 128).transpose(3, 0, 2, 1)
        .reshape(128, NSC * KC * IT).astype(ml_dtypes.bfloat16))


def _pack_w(w):
    # [1024, 256] -> [128, 8*256] chunk-major
    return np.ascontiguousarray(
        w.reshape(KC, 128, DG).transpose(1, 0, 2)
        .reshape(128, KC * DG).astype(ml_dtypes.bfloat16))


def _pack_wo(w):
    # [256, 1024] -> [128, 2*1024] pair-major
    return np.ascontiguousarray(
        w.reshape(2, 128, D).transpose(1, 0, 2)
        .reshape(128, 2 * D).astype(ml_dtypes.bfloat16))


def _in_maps(x, Wq, Wk, Wv, Wo):
    maps = []
    ones = np.ones((128, 128), ml_dtypes.bfloat16)
    for c in range(NC):
        b, g = c // (NC // B), c % (NC // B)
        wqp = _pack_w(Wq[:, g * DG:(g + 1) * DG])
        wkp = _pack_w(Wk[:, g * DG:(g + 1) * DG])
        maps.append({
            "xp": _pack_x(x[b]),
            "wq": wqp,
            "wk": wkp,
            "wv": _pack_w(Wv[:, g * DG:(g + 1) * DG]),
            "wo": _pack_wo(Wo[g * DG:(g + 1) * DG, :]),
            "wf": np.ascontiguousarray(
                np.concatenate([wqp[:, 0:DG], wkp[:, 0:DG], ones], axis=1)),
        })
    return maps


def run(x, Wq, Wk, Wv, Wo, bo, trace=False):
    if "nc" not in _CACHE:
        _CACHE["nc"] = _build()
    nc = _CACHE["nc"]
    res = bass_utils.run_bass_kernel_spmd(
        nc, _in_maps(x, Wq, Wk, Wv, Wo), core_ids=list(range(NC)), trace=trace,
    )
    parts = [res.results[c]["out"] for c in range(NC)]
    gpb = NC // B
    full = np.stack([sum(parts[b * gpb + 1: (b + 1) * gpb], parts[b * gpb]) for b in range(B)])
    full = full + np.asarray(bo, np.float32)[None, None, :]
    return full.astype(np.float32), res


def kernel(x, Wq, Wk, Wv, Wo, bo):
    x = np.asarray(x, np.float32)
    full, _ = run(x, np.asarray(Wq, np.float32), np.asarray(Wk, np.float32),
                  np.asarray(Wv, np.float32), np.asarray(Wo, np.float32),
                  np.asarray(bo, np.float32))
    return full
